# revision 11
# baseline (speedup 1.0000x reference)
"""Trainium2 Bass kernel for nn_CrossAttention_5385888989393.

Contract: kernel(**inputs) takes FULL inputs (batch 8) and returns the FULL
output, sharding batch-parallel across 8 NeuronCores (1 batch element per
core, no collectives).

Algorithm per batch (channel attention, contraction over spatial n=4096):
    G     = f_m @ f_n^T                     [512, 512]  Gram over n
    T2T   = G^T @ Wq^T                      [512, 512]  (G stationary)
    D^T_h = Wk_h-contraction with T2T       [64, 64] per head (diag tiles)
    E^T   = exp(D^T * scale) * headmask     (softmax numerator, transposed)
    SE_h  = E_h @ Wv_h   (via lhsT = E^T)   [64, 512]
    S_h   = SE_h / rowsum(E_h)              (deferred softmax normalization)
    M^T   = S-contraction with Wout^T       [512, 512]
    out   = (M @ f_n) + bout                [512, 4096]

~2x fewer FLOPs than the naive q/k/v path (spatial dim collapses through
the Gram immediately).

Dataflow (from NTFF traces of prior versions): everything device-side is
bf16 except the PSUM accumulators and the softmax correction scalars
(rel err ~7e-3 vs the 2e-2 gate). The host ships f_m/f_n/weights to device
DRAM already in bf16: the device would otherwise burn ~35us of ACT/DVE
converting, fp32r/fp32 staging disables the PE's fast-weight-load path
(fp32r measured 240ns per 128-col transpose vs ~95ns bf16), and input DMA
traffic halves. The Gram operands are transposed on the PE (the DMA XBAR
transpose path measured only ~73GB/s from DRAM - far too slow). The
output returns as bf16 (upcast on host), halving store traffic. DMA
dispatch costs ~620ns of sequencer time per dma_start, so transfers are
few and wide ([128,1024] quarters for the inputs, one [p,(t c)] DMA per
weight matrix, one packed constant array, [128,1024] output pairs: ~57
DMAs vs 119 in the fp32 version). A DMA-free warm-up (transposes of a
memset tile) ramps the PE p-state from ~3us while the first load lands.
"""
import sys

if "/opt/trn_rl_repo" not in sys.path:
    sys.path.insert(0, "/opt/trn_rl_repo")

import numpy as np
import ml_dtypes

import concourse.bass as bass
import concourse.tile as tile
from concourse import bacc, mybir
from concourse.bass_utils import run_bass_kernel_spmd
F32 = mybir.dt.float32
BF16 = mybir.dt.bfloat16
EXP = mybir.ActivationFunctionType.Exp
CP = mybir.ActivationFunctionType.Copy
IDENT_FN = mybir.ActivationFunctionType.Identity

P = 128          # partitions
C = 512          # channels
CT = C // P      # 4 channel tiles
NN = 4096        # spatial (64*64)
NCH = NN // 512  # 8 column chunks of 512
NSUB = NN // P   # 32 column subchunks of 128
DH = 64
SCALE = DH ** -0.5
B = 8            # batch == n_cores

_CACHED_NC = None
_CACHED_RUNNER = None

_DMASK = np.kron(np.eye(2, dtype=np.float32), np.ones((DH, DH), np.float32))


def _build():
    nc = bacc.Bacc("TRN2", target_bir_lowering=False, debug=False, num_devices=B)

    fm_d = nc.dram_tensor("f_mb", [C, NN], BF16, kind="ExternalInput").ap()
    fn_d = nc.dram_tensor("f_nb", [C, NN], BF16, kind="ExternalInput").ap()
    wqt_d = nc.dram_tensor("WqT", [C, C], BF16, kind="ExternalInput").ap()
    wkt_d = nc.dram_tensor("WkT", [C, C], BF16, kind="ExternalInput").ap()
    wv_d = nc.dram_tensor("Wv", [C, C], BF16, kind="ExternalInput").ap()
    woutt_d = nc.dram_tensor("WoutT", [C, C], BF16, kind="ExternalInput").ap()
    # packed [128, 260]: identity | head-mask | bout as 4 columns
    cst_d = nc.dram_tensor("cst", [P, 260], F32, kind="ExternalInput").ap()
    out_d = nc.dram_tensor("out", [C, NN], BF16, kind="ExternalOutput").ap()

    with tile.TileContext(nc) as tc:
        with (
            tc.tile_pool(name="const", bufs=1) as const,
            tc.tile_pool(name="w", bufs=1) as wpool,
            tc.tile_pool(name="ft", bufs=3) as ftpool,
            tc.tile_pool(name="fmb", bufs=1) as fmbpool,
            tc.tile_pool(name="small", bufs=1) as small,
            tc.tile_pool(name="fnb", bufs=1) as fnbpool,
            tc.tile_pool(name="outst", bufs=2) as outst,
            tc.tile_pool(name="gacc", bufs=1, space="PSUM") as gacc,
            tc.tile_pool(name="work", bufs=2, space="PSUM") as work,
        ):
            # ---------- DMA-free warm-up ------------------------------------
            # HAM warm-up: back-to-back transposes of a memset tile fill the
            # otherwise PE-idle startup window (waiting on the first data
            # chunk) with sustained PE activity, so the first real matmuls
            # run at 2.4 GHz instead of the cold 1.2 GHz. The written values
            # are garbage and never read; ordering vs the first Gram matmul
            # comes from the WAW dep on the wk0 tile.
            warmsrc = const.tile([P, P], BF16, tag="warmsrc")
            nc.vector.memset(warmsrc[:], 1.0)
            warm_ps = work.tile([P, 2, C], BF16, tag="wk0", name="warmps")
            for i in range(10):
                wsl = slice((i % 4) * P, ((i % 4) + 1) * P)
                nc.tensor.transpose(warm_ps[:, 0, wsl], warmsrc[:],
                                    warmsrc[:])

            # ones rhs for the rowsum matmuls (exact in bf16)
            ones2_b = const.tile([P, 2], BF16, tag="ones2_b")
            nc.vector.memset(ones2_b[:], 1.0)

            # ---------- packed constants ------------------------------------
            cst = const.tile([P, 260], F32, tag="cst")
            nc.sync.dma_start(cst[:], cst_d)
            identb = const.tile([P, P], BF16, tag="identb")
            nc.vector.tensor_copy(identb[:], cst[:, 0:P])
            # block-diagonal 0/1 mask to zero cross-head blocks of E^T
            dmask = const.tile([P, P], BF16, tag="dmask")
            nc.vector.tensor_copy(dmask[:], cst[:, P:2 * P])
            bout_sb = [cst[:, 2 * P + ct:2 * P + ct + 1] for ct in range(CT)]

            # ---------- phase 1: Gram accumulation over 32 subchunks --------
            # Inputs arrive bf16 in [128,1024] quarters; both Gram operands
            # are transposed on the PE (FWL makes bf16 transposes ~95ns).
            g_ps = [
                gacc.tile([P, C], F32, tag=f"g{at}", name=f"g_ps{at}")
                for at in range(CT)
            ]

            # interleave f_m/f_n quarter loads: q0 of everything first so the
            # first transposes start as early as possible
            fmq = [[None] * 4 for _ in range(CT)]
            fnq = [[None] * 4 for _ in range(CT)]
            for q in range(4):
                qsl = slice(q * 1024, (q + 1) * 1024)
                for ct in range(CT):
                    t = fmbpool.tile([P, 1024], BF16, tag=f"fmq{ct}_{q}",
                                     name=f"fmq{ct}_{q}")
                    nc.sync.dma_start(t[:], fm_d[ct * P:(ct + 1) * P, qsl])
                    fmq[ct][q] = t
                    t = fnbpool.tile([P, 1024], BF16, tag=f"fnq{ct}_{q}",
                                     name=f"fnq{ct}_{q}")
                    # q0 f_n loads dispatch from the (still-idle) ACT queue so
                    # the first su-pair isn't gated on 9 serial SP dispatches
                    eng = nc.scalar if q == 0 else nc.sync
                    eng.dma_start(t[:], fn_d[ct * P:(ct + 1) * P, qsl])
                    fnq[ct][q] = t

            for ch in range(NCH):
                q, qoff = ch // 2, (ch % 2) * 512
                for sp in range(2):
                    # transpose an f_m/f_n su-PAIR -> [n128, 2, c512] (bf16);
                    # one wide PSUM evacuation per operand halves the copy
                    # count and the cross-engine handoffs
                    tpm = work.tile([P, 2, C], BF16, tag="wk0", name="tpm")
                    tpn = work.tile([P, 2, C], BF16, tag="wk1", name="tpn")
                    for su2 in range(2):
                        su = sp * 2 + su2
                        sl = slice(qoff + su * P, qoff + (su + 1) * P)
                        for ct in range(CT):
                            nc.tensor.transpose(
                                tpm[:, su2, ct * P:(ct + 1) * P],
                                fmq[ct][q][:, sl], identb[:]
                            )
                        for ct in range(CT):
                            nc.tensor.transpose(
                                tpn[:, su2, ct * P:(ct + 1) * P],
                                fnq[ct][q][:, sl], identb[:]
                            )
                    fmT = ftpool.tile([P, 2, C], BF16, tag="fmT")
                    nc.vector.tensor_copy(fmT[:], tpm[:])
                    fnT = ftpool.tile([P, 2, C], BF16, tag="fnT")
                    nc.scalar.activation(fnT[:], tpn[:], CP)

                    # Gram: G[a-tile, :] += fmT[su][:, a-tile].T @ fnT[su]
                    for su2 in range(2):
                        s = ch * 4 + sp * 2 + su2
                        for at in range(CT):
                            nc.tensor.matmul(
                                g_ps[at][:],
                                fmT[:, su2, at * P:(at + 1) * P],
                                fnT[:, su2, :],
                                start=(s == 0),
                                stop=(s == NSUB - 1),
                            )

            def load_w(dram, name):
                st = wpool.tile([P, CT, C], BF16, tag=f"w{name}")
                nc.sync.dma_start(
                    st[:], dram.rearrange("(t p) c -> p t c", p=P)
                )
                return [st[:, rt, :] for rt in range(CT)]

            WqT = load_w(wqt_d, "wqT")      # WqT[a][., (h,i)]
            WkT = load_w(wkt_d, "wkT")      # WkT[b][., (h,j)]
            WoutT = load_w(woutt_d, "woT")  # WoutT[e][., o]
            Wv_b = load_w(wv_d, "wv")       # Wv rows (h,j), cols c

            G_sb = []
            for at in range(CT):
                g = small.tile([P, C], BF16, tag=f"G{at}")
                if at < 2:
                    nc.vector.tensor_copy(g[:], g_ps[at][:])
                else:
                    nc.scalar.activation(g[:], g_ps[at][:], CP)
                G_sb.append(g)

            # ---------- phase 2: logits, softmax, value mixing ------------
            # T2T[b, (h,i)] = sum_a G[a, b] * WqT[a, (h,i)]
            # (G natural as stationary -> transposed product for free)
            T2T_sb = []
            for bt in range(CT):
                ps = work.tile([P, C], F32, tag="wk1", name="t2tps")
                for at in range(CT):
                    nc.tensor.matmul(
                        ps[:],
                        G_sb[at][:, bt * P:(bt + 1) * P],
                        WqT[at],
                        start=(at == 0),
                        stop=(at == CT - 1),
                    )
                t = small.tile([P, C], BF16, tag=f"T2T_{bt}")
                if bt < 2:
                    nc.vector.tensor_copy(t[:], ps[:])
                else:
                    nc.scalar.activation(t[:], ps[:], CP)
                T2T_sb.append(t)

            # Diagonal head-pair tiles of D^T = Wk @ T2T ; E^T = exp(scale*D^T)
            ET = []
            for jt in range(CT):
                sl = slice(jt * P, (jt + 1) * P)
                ps = work.tile([P, P], F32, tag="wk0", name="dps")
                for bt in range(CT):
                    nc.tensor.matmul(
                        ps[:], WkT[bt][:, sl], T2T_sb[bt][:, sl],
                        start=(bt == 0), stop=(bt == CT - 1),
                    )
                etmp = small.tile([P, P], BF16, tag="etmp")
                nc.scalar.activation(etmp[:], ps[:], EXP, scale=SCALE)
                e = small.tile([P, P], BF16, tag=f"G{jt}", name=f"ET{jt}")
                # zero the cross-head blocks so full-width matmuls (SE,
                # rowsums) see exact per-head separation
                nc.vector.tensor_mul(e[:], etmp[:], dmask[:])
                ET.append(e)

            # rowsums r[(h,i)] = sum_j E_h[i, j]
            inv_sb = []
            for it in range(CT):
                rps = work.tile([P, 2], F32, tag="wk1", name="rps")
                nc.tensor.matmul(rps[:], ET[it][:], ones2_b[:], start=True,
                                 stop=True)
                inv = small.tile([P, 1], F32, tag=f"inv{it}")
                nc.vector.reciprocal(inv[:], rps[:, 0:1])
                inv_sb.append(inv)

            # SE_h = E_h @ Wv_h ; S = SE * inv_r (deferred softmax division)
            S_sb = []
            for it in range(CT):
                seps = work.tile([P, C], F32, tag="wk0", name="seps")
                nc.tensor.matmul(
                    seps[:], ET[it][:], Wv_b[it], start=True, stop=True,
                )
                s_t = small.tile([P, C], BF16, tag=f"S{it}", name=f"S{it}")
                if it < 2:
                    nc.vector.tensor_scalar_mul(s_t[:], seps[:], inv_sb[it][:])
                else:
                    nc.scalar.activation(s_t[:], seps[:], CP,
                                         scale=inv_sb[it][:])
                S_sb.append(s_t)

            # M^T[c, o] = sum_e S[e][:, c] * WoutT[e][:, o]
            MT_sb = []
            for ct in range(CT):
                ps = work.tile([P, C], F32, tag="wk1", name="mtps")
                for et in range(CT):
                    nc.tensor.matmul(
                        ps[:],
                        S_sb[et][:, ct * P:(ct + 1) * P],
                        WoutT[et],
                        start=(et == 0),
                        stop=(et == CT - 1),
                    )
                t = small.tile([P, C], BF16, tag=f"T2T_{ct}", name=f"MT{ct}")
                if ct < 2:
                    nc.vector.tensor_copy(t[:], ps[:])
                else:
                    nc.scalar.activation(t[:], ps[:], CP)
                MT_sb.append(t)

            # ---------- phase 3: out = M @ f_n + bout; bf16 stores ----------
            opair = {}
            for ch in range(NCH):
                for ot in range(CT):
                    slot = (ch * CT + ot) % 6
                    if slot < 4:
                        ps = gacc.tile([P, 512], F32, tag=f"g{slot}",
                                       name=f"ops{ch}_{ot}")
                    else:
                        ps = work.tile([P, 512], F32, tag=f"wk{slot - 4}",
                                       name=f"ops{ch}_{ot}")
                    for ct in range(CT):
                        nc.tensor.matmul(
                            ps[:],
                            MT_sb[ct][:, ot * P:(ot + 1) * P],
                            fnq[ct][ch // 2][:, (ch % 2) * 512:
                                             (ch % 2) * 512 + 512],
                            start=(ct == 0),
                            stop=(ct == CT - 1),
                        )
                    paired = ch < 6
                    if paired and ch % 2 == 0:
                        opair[ot] = outst.tile([P, 1024], BF16, tag=f"out{ot}",
                                               name=f"opair{ot}_{ch}")
                    elif not paired:
                        opair[ot] = outst.tile([P, 1024], BF16, tag=f"out{ot}",
                                               name=f"osing{ot}_{ch}")
                    o = opair[ot]
                    hsl = slice((ch % 2) * 512, (ch % 2) * 512 + 512) \
                        if paired else slice(0, 512)
                    # split evacuation between ACT and DVE so neither
                    # serializes the drain (esp. for the final chunk)
                    if ot % 2 == 1:
                        nc.scalar.activation(o[:, hsl], ps[:], IDENT_FN,
                                             bias=bout_sb[ot])
                    else:
                        nc.vector.tensor_scalar_add(o[:, hsl], ps[:],
                                                    bout_sb[ot])
                    if paired and ch % 2 == 1:
                        nc.sync.dma_start(
                            out_d[ot * P:(ot + 1) * P,
                                  (ch - 1) * 512:(ch + 1) * 512],
                            o[:],
                        )
                    elif not paired:
                        nc.sync.dma_start(
                            out_d[ot * P:(ot + 1) * P,
                                  ch * 512:(ch + 1) * 512],
                            o[:, 0:512],
                        )

    nc.compile()
    return nc


def _get_nc():
    global _CACHED_NC
    if _CACHED_NC is None:
        _CACHED_NC = _build()
    return _CACHED_NC


def _get_runner():
    """Memoized PJRT runner: jax.jit-compiled once, reused across kernel()
    calls (run_bass_kernel_spmd rebuilds the jit closure every call, which
    forces a ~minute-long recompile)."""
    global _CACHED_RUNNER
    if _CACHED_RUNNER is not None:
        return _CACHED_RUNNER

    import jax
    from jax.sharding import Mesh, PartitionSpec
    from jax.experimental.shard_map import shard_map
    import concourse.mybir as mybir_
    from concourse.bass2jax import (
        _bass_exec_p,
        install_neuronx_cc_hook,
        partition_id_tensor,
    )

    nc = _get_nc()
    install_neuronx_cc_hook()

    partition_name = (
        nc.partition_id_tensor.name if nc.partition_id_tensor else None
    )
    in_names = []
    out_names = []
    out_avals = []
    out_shapes = []
    for alloc in nc.m.functions[0].allocations:
        if not isinstance(alloc, mybir_.MemoryLocationSet):
            continue
        name = alloc.memorylocations[0].name
        if alloc.kind == "ExternalInput":
            if name != partition_name:
                in_names.append(name)
        elif alloc.kind == "ExternalOutput":
            shape = tuple(alloc.tensor_shape)
            dtype = mybir_.dt.np(alloc.dtype)
            out_names.append(name)
            out_avals.append(jax.core.ShapedArray(shape, dtype))
            out_shapes.append((shape, dtype))
    n_params = len(in_names)
    n_outs = len(out_names)
    all_names = tuple(in_names + out_names)
    if partition_name is not None:
        all_names = all_names + (partition_name,)
    donate = tuple(range(n_params, n_params + n_outs))

    def _body(*args):
        operands = list(args)
        if partition_name is not None:
            operands.append(partition_id_tensor())
        outs = _bass_exec_p.bind(
            *operands,
            out_avals=tuple(out_avals),
            in_names=all_names,
            out_names=tuple(out_names),
            lowering_input_output_aliases=(),
            sim_require_finite=True,
            sim_require_nnan=True,
            nc=nc,
        )
        return tuple(outs)

    devices = jax.devices()[:B]
    mesh = Mesh(np.asarray(devices), ("core",))
    sharded = jax.jit(
        shard_map(
            _body,
            mesh=mesh,
            in_specs=(PartitionSpec("core"),) * (n_params + n_outs),
            out_specs=(PartitionSpec("core"),) * n_outs,
            check_rep=False,
        ),
        donate_argnums=donate,
        keep_unused=True,
    )

    def run(in_maps):
        concat_in = [
            np.concatenate([np.asarray(m[k]) for m in in_maps], axis=0)
            for k in in_names
        ]
        concat_zeros = [
            np.zeros((B * s[0], *s[1:]), dt) for (s, dt) in out_shapes
        ]
        out_arrs = sharded(*concat_in, *concat_zeros)
        return [
            {
                k: np.asarray(out_arrs[i]).reshape(B, *out_shapes[i][0])[c]
                for i, k in enumerate(out_names)
            }
            for c in range(B)
        ]

    _CACHED_RUNNER = run
    return run


def kernel(f_m, f_n, Wq, Wkv, Wout, bout, trace=False):
    f_m = np.asarray(f_m, dtype=np.float32)
    f_n = np.asarray(f_n, dtype=np.float32)
    Wq = np.asarray(Wq, dtype=np.float32)
    Wkv = np.asarray(Wkv, dtype=np.float32)
    Wout = np.asarray(Wout, dtype=np.float32)
    bout = np.asarray(bout, dtype=np.float32)

    b, c, h, w = f_m.shape
    nc = _get_nc()
    bf = ml_dtypes.bfloat16
    fm_b = np.ascontiguousarray(f_m.reshape(b, C, NN).astype(bf))
    fn_b = np.ascontiguousarray(f_n.reshape(b, C, NN).astype(bf))
    wqt = np.ascontiguousarray(Wq.T.astype(bf))
    wkt = np.ascontiguousarray(Wkv[:C].T.astype(bf))
    wv = np.ascontiguousarray(Wkv[C:].astype(bf))
    woutt = np.ascontiguousarray(Wout.T.astype(bf))
    cst = np.ascontiguousarray(
        np.concatenate(
            [np.eye(P, dtype=np.float32), _DMASK, bout.reshape(CT, P).T],
            axis=1,
        ).astype(np.float32)
    )
    in_maps = [
        {
            "f_mb": fm_b[i],
            "f_nb": fn_b[i],
            "WqT": wqt,
            "WkT": wkt,
            "Wv": wv,
            "WoutT": woutt,
            "cst": cst,
        }
        for i in range(b)
    ]
    if trace:
        res = run_bass_kernel_spmd(
            nc, in_maps, core_ids=list(range(B)), trace=True
        )
        kernel.last_results = res
        results = res.results
    else:
        results = _get_runner()(in_maps)
    return np.stack(
        [r["out"].astype(np.float32).reshape(c, h, w) for r in results]
    )


# revision 12
# speedup vs baseline: 1.0114x; 1.0114x over previous
"""Trainium2 Bass kernel for nn_CrossAttention_5385888989393.

Contract: kernel(**inputs) takes FULL inputs (batch 8) and returns the FULL
output, sharding batch-parallel across 8 NeuronCores (1 batch element per
core, no collectives).

Algorithm per batch (channel attention, contraction over spatial n=4096):
    G     = f_m @ f_n^T                     [512, 512]  Gram over n
    T2T   = G^T @ Wq^T                      [512, 512]  (G stationary)
    D^T_h = Wk_h-contraction with T2T       [64, 64] per head (diag tiles)
    E^T   = exp(D^T * scale) * headmask     (softmax numerator, transposed)
    SE_h  = E_h @ Wv_h   (via lhsT = E^T)   [64, 512]
    S_h   = SE_h / rowsum(E_h)              (deferred softmax normalization)
    M^T   = S-contraction with Wout^T       [512, 512]
    out   = (M @ f_n) + bout                [512, 4096]

~2x fewer FLOPs than the naive q/k/v path (spatial dim collapses through
the Gram immediately).

Dataflow (from NTFF traces of prior versions): everything device-side is
bf16 except the PSUM accumulators and the softmax correction scalars
(rel err ~7e-3 vs the 2e-2 gate). The host ships f_m/f_n/weights to device
DRAM already in bf16: the device would otherwise burn ~35us of ACT/DVE
converting, fp32r/fp32 staging disables the PE's fast-weight-load path
(fp32r measured 240ns per 128-col transpose vs ~95ns bf16), and input DMA
traffic halves. The Gram operands are transposed on the PE (the DMA XBAR
transpose path measured only ~73GB/s from DRAM - far too slow). The
output returns as bf16 (upcast on host), halving store traffic. DMA
dispatch costs ~620ns of sequencer time per dma_start, so transfers are
few and wide ([128,1024] quarters for the inputs, one [p,(t c)] DMA per
weight matrix, one packed constant array, [128,1024] output pairs: ~57
DMAs vs 119 in the fp32 version). A DMA-free warm-up (transposes of a
memset tile) ramps the PE p-state from ~3us while the first load lands.
"""
import sys

if "/opt/trn_rl_repo" not in sys.path:
    sys.path.insert(0, "/opt/trn_rl_repo")

import numpy as np
import ml_dtypes

import concourse.bass as bass
import concourse.tile as tile
from concourse import bacc, mybir
from concourse.bass_utils import run_bass_kernel_spmd
F32 = mybir.dt.float32
BF16 = mybir.dt.bfloat16
EXP = mybir.ActivationFunctionType.Exp
CP = mybir.ActivationFunctionType.Copy
IDENT_FN = mybir.ActivationFunctionType.Identity

P = 128          # partitions
C = 512          # channels
CT = C // P      # 4 channel tiles
NN = 4096        # spatial (64*64)
NCH = NN // 512  # 8 column chunks of 512
NSUB = NN // P   # 32 column subchunks of 128
DH = 64
SCALE = DH ** -0.5
B = 8            # batch == n_cores

_CACHED_NC = None
_CACHED_RUNNER = None

_DMASK = np.kron(np.eye(2, dtype=np.float32), np.ones((DH, DH), np.float32))


def _build():
    nc = bacc.Bacc("TRN2", target_bir_lowering=False, debug=False, num_devices=B)

    fm_d = nc.dram_tensor("f_mb", [C, NN], BF16, kind="ExternalInput").ap()
    fn_d = nc.dram_tensor("f_nb", [C, NN], BF16, kind="ExternalInput").ap()
    wqt_d = nc.dram_tensor("WqT", [C, C], BF16, kind="ExternalInput").ap()
    wkt_d = nc.dram_tensor("WkT", [C, C], BF16, kind="ExternalInput").ap()
    wv_d = nc.dram_tensor("Wv", [C, C], BF16, kind="ExternalInput").ap()
    woutt_d = nc.dram_tensor("WoutT", [C, C], BF16, kind="ExternalInput").ap()
    # packed [128, 260]: identity | head-mask | bout as 4 columns
    cst_d = nc.dram_tensor("cst", [P, 260], F32, kind="ExternalInput").ap()
    out_d = nc.dram_tensor("out", [C, NN], BF16, kind="ExternalOutput").ap()

    with tile.TileContext(nc) as tc:
        with (
            tc.tile_pool(name="const", bufs=1) as const,
            tc.tile_pool(name="w", bufs=1) as wpool,
            tc.tile_pool(name="ft", bufs=3) as ftpool,
            tc.tile_pool(name="fmb", bufs=1) as fmbpool,
            tc.tile_pool(name="small", bufs=1) as small,
            tc.tile_pool(name="fnb", bufs=1) as fnbpool,
            tc.tile_pool(name="outst", bufs=2) as outst,
            tc.tile_pool(name="gacc", bufs=1, space="PSUM") as gacc,
            tc.tile_pool(name="work", bufs=2, space="PSUM") as work,
        ):
            # ---------- DMA-free warm-up ------------------------------------
            # HAM warm-up: back-to-back transposes of a memset tile fill the
            # otherwise PE-idle startup window (waiting on the first data
            # chunk) with sustained PE activity, so the first real matmuls
            # run at 2.4 GHz instead of the cold 1.2 GHz. The written values
            # are garbage and never read; ordering vs the first Gram matmul
            # comes from the WAW dep on the wk0 tile.
            warmsrc = const.tile([P, P], BF16, tag="warmsrc")
            nc.vector.memset(warmsrc[:], 1.0)
            warm_ps = work.tile([P, 2, C], BF16, tag="wk0", name="warmps")
            for i in range(10):
                wsl = slice((i % 4) * P, ((i % 4) + 1) * P)
                nc.tensor.transpose(warm_ps[:, 0, wsl], warmsrc[:],
                                    warmsrc[:])

            # ones rhs for the rowsum matmuls (exact in bf16)
            ones2_b = const.tile([P, 2], BF16, tag="ones2_b")
            nc.vector.memset(ones2_b[:], 1.0)

            # ---------- packed constants ------------------------------------
            cst = const.tile([P, 260], F32, tag="cst")
            nc.sync.dma_start(cst[:], cst_d)
            identb = const.tile([P, P], BF16, tag="identb")
            nc.vector.tensor_copy(identb[:], cst[:, 0:P])
            # block-diagonal 0/1 mask to zero cross-head blocks of E^T
            dmask = const.tile([P, P], BF16, tag="dmask")
            nc.vector.tensor_copy(dmask[:], cst[:, P:2 * P])
            bout_sb = [cst[:, 2 * P + ct:2 * P + ct + 1] for ct in range(CT)]

            # ---------- phase 1: Gram accumulation over 32 subchunks --------
            # Inputs arrive bf16 in [128,1024] quarters; both Gram operands
            # are transposed on the PE (FWL makes bf16 transposes ~95ns).
            g_ps = [
                gacc.tile([P, C], F32, tag=f"g{at}", name=f"g_ps{at}")
                for at in range(CT)
            ]

            # interleave f_m/f_n quarter loads: q0 of everything first so the
            # first transposes start as early as possible
            fmq = [[None] * 4 for _ in range(CT)]
            fnq = [[None] * 4 for _ in range(CT)]
            for q in range(4):
                qsl = slice(q * 1024, (q + 1) * 1024)
                for ct in range(CT):
                    t = fmbpool.tile([P, 1024], BF16, tag=f"fmq{ct}_{q}",
                                     name=f"fmq{ct}_{q}")
                    nc.sync.dma_start(t[:], fm_d[ct * P:(ct + 1) * P, qsl])
                    fmq[ct][q] = t
                    t = fnbpool.tile([P, 1024], BF16, tag=f"fnq{ct}_{q}",
                                     name=f"fnq{ct}_{q}")
                    # q0 f_n loads dispatch from the (still-idle) ACT queue so
                    # the first su-pair isn't gated on 9 serial SP dispatches
                    eng = nc.scalar if q == 0 else nc.sync
                    eng.dma_start(t[:], fn_d[ct * P:(ct + 1) * P, qsl])
                    fnq[ct][q] = t

            for ch in range(NCH):
                q, qoff = ch // 2, (ch % 2) * 512
                for sp in range(2):
                    # transpose an f_m/f_n su-PAIR -> [n128, 2, c512] (bf16);
                    # one wide PSUM evacuation per operand halves the copy
                    # count and the cross-engine handoffs
                    tpm = work.tile([P, 2, C], BF16, tag="wk0", name="tpm")
                    tpn = work.tile([P, 2, C], BF16, tag="wk1", name="tpn")
                    for su2 in range(2):
                        su = sp * 2 + su2
                        sl = slice(qoff + su * P, qoff + (su + 1) * P)
                        for ct in range(CT):
                            nc.tensor.transpose(
                                tpm[:, su2, ct * P:(ct + 1) * P],
                                fmq[ct][q][:, sl], identb[:]
                            )
                        for ct in range(CT):
                            nc.tensor.transpose(
                                tpn[:, su2, ct * P:(ct + 1) * P],
                                fnq[ct][q][:, sl], identb[:]
                            )
                    fmT = ftpool.tile([P, 2, C], BF16, tag="fmT")
                    nc.vector.tensor_copy(fmT[:], tpm[:])
                    fnT = ftpool.tile([P, 2, C], BF16, tag="fnT")
                    nc.scalar.activation(fnT[:], tpn[:], CP)

                    # Gram: G[a-tile, :] += fmT[su][:, a-tile].T @ fnT[su]
                    for su2 in range(2):
                        s = ch * 4 + sp * 2 + su2
                        for at in range(CT):
                            nc.tensor.matmul(
                                g_ps[at][:],
                                fmT[:, su2, at * P:(at + 1) * P],
                                fnT[:, su2, :],
                                start=(s == 0),
                                stop=(s == NSUB - 1),
                            )

            def load_w(dram, name):
                st = wpool.tile([P, CT, C], BF16, tag=f"w{name}")
                nc.sync.dma_start(
                    st[:], dram.rearrange("(t p) c -> p t c", p=P)
                )
                return [st[:, rt, :] for rt in range(CT)]

            WqT = load_w(wqt_d, "wqT")      # WqT[a][., (h,i)]
            WkT = load_w(wkt_d, "wkT")      # WkT[b][., (h,j)]
            WoutT = load_w(woutt_d, "woT")  # WoutT[e][., o]
            Wv_b = load_w(wv_d, "wv")       # Wv rows (h,j), cols c

            G_sb = []
            for at in range(CT):
                g = small.tile([P, C], BF16, tag=f"G{at}")
                if at < 2:
                    nc.vector.tensor_copy(g[:], g_ps[at][:])
                else:
                    nc.scalar.activation(g[:], g_ps[at][:], CP)
                G_sb.append(g)

            # ---------- phase 2: logits, softmax, value mixing ------------
            # T2T[b, (h,i)] = sum_a G[a, b] * WqT[a, (h,i)]
            # (G natural as stationary -> transposed product for free)
            T2T_sb = []
            for bt in range(CT):
                ps = work.tile([P, C], F32, tag="wk1", name="t2tps")
                for at in range(CT):
                    nc.tensor.matmul(
                        ps[:],
                        G_sb[at][:, bt * P:(bt + 1) * P],
                        WqT[at],
                        start=(at == 0),
                        stop=(at == CT - 1),
                    )
                t = small.tile([P, C], BF16, tag=f"T2T_{bt}")
                if bt < 2:
                    nc.vector.tensor_copy(t[:], ps[:])
                else:
                    nc.scalar.activation(t[:], ps[:], CP)
                T2T_sb.append(t)

            # Diagonal head-pair tiles of D^T = Wk @ T2T ; E^T = exp(scale*D^T)
            ET = []
            for jt in range(CT):
                sl = slice(jt * P, (jt + 1) * P)
                ps = work.tile([P, P], F32, tag="wk0", name="dps")
                for bt in range(CT):
                    nc.tensor.matmul(
                        ps[:], WkT[bt][:, sl], T2T_sb[bt][:, sl],
                        start=(bt == 0), stop=(bt == CT - 1),
                    )
                etmp = small.tile([P, P], BF16, tag="etmp")
                nc.scalar.activation(etmp[:], ps[:], EXP, scale=SCALE)
                e = small.tile([P, P], BF16, tag=f"G{jt}", name=f"ET{jt}")
                # zero the cross-head blocks so full-width matmuls (SE,
                # rowsums) see exact per-head separation
                nc.vector.tensor_mul(e[:], etmp[:], dmask[:])
                ET.append(e)

            # rowsums r[(h,i)] = sum_j E_h[i, j]
            inv_sb = []
            for it in range(CT):
                rps = work.tile([P, 2], F32, tag="wk1", name="rps")
                nc.tensor.matmul(rps[:], ET[it][:], ones2_b[:], start=True,
                                 stop=True)
                inv = small.tile([P, 1], F32, tag=f"inv{it}")
                nc.vector.reciprocal(inv[:], rps[:, 0:1])
                inv_sb.append(inv)

            # SE_h = E_h @ Wv_h ; S = SE * inv_r (deferred softmax division)
            S_sb = []
            for it in range(CT):
                seps = work.tile([P, C], F32, tag="wk0", name="seps")
                nc.tensor.matmul(
                    seps[:], ET[it][:], Wv_b[it], start=True, stop=True,
                )
                s_t = small.tile([P, C], BF16, tag=f"S{it}", name=f"S{it}")
                if it < 2:
                    nc.vector.tensor_scalar_mul(s_t[:], seps[:], inv_sb[it][:])
                else:
                    nc.scalar.activation(s_t[:], seps[:], CP,
                                         scale=inv_sb[it][:])
                S_sb.append(s_t)

            # M^T[c, o] = sum_e S[e][:, c] * WoutT[e][:, o]
            MT_sb = []
            for ct in range(CT):
                ps = work.tile([P, C], F32, tag="wk1", name="mtps")
                for et in range(CT):
                    nc.tensor.matmul(
                        ps[:],
                        S_sb[et][:, ct * P:(ct + 1) * P],
                        WoutT[et],
                        start=(et == 0),
                        stop=(et == CT - 1),
                    )
                t = small.tile([P, C], BF16, tag=f"T2T_{ct}", name=f"MT{ct}")
                if ct < 2:
                    nc.vector.tensor_copy(t[:], ps[:])
                else:
                    nc.scalar.activation(t[:], ps[:], CP)
                MT_sb.append(t)

            # ---------- phase 3: out = M @ f_n + bout; bf16 stores ----------
            opair = {}
            for ch in range(NCH):
                for ot in range(CT):
                    idx = ch * CT + ot
                    slot = idx % 4 if ch < 2 else (idx - 8) % 6
                    if slot < 4:
                        ps = gacc.tile([P, 512], F32, tag=f"g{slot}",
                                       name=f"ops{ch}_{ot}")
                    else:
                        ps = work.tile([P, 512], F32, tag=f"wk{slot - 4}",
                                       name=f"ops{ch}_{ot}")
                    for ct in range(CT):
                        nc.tensor.matmul(
                            ps[:],
                            MT_sb[ct][:, ot * P:(ot + 1) * P],
                            fnq[ct][ch // 2][:, (ch % 2) * 512:
                                             (ch % 2) * 512 + 512],
                            start=(ct == 0),
                            stop=(ct == CT - 1),
                        )
                    paired = ch < 6
                    if paired and ch % 2 == 0:
                        opair[ot] = outst.tile([P, 1024], BF16, tag=f"out{ot}",
                                               name=f"opair{ot}_{ch}")
                    elif not paired:
                        opair[ot] = outst.tile([P, 1024], BF16, tag=f"out{ot}",
                                               name=f"osing{ot}_{ch}")
                    o = opair[ot]
                    hsl = slice((ch % 2) * 512, (ch % 2) * 512 + 512) \
                        if paired else slice(0, 512)
                    # split evacuation between ACT and DVE so neither
                    # serializes the drain (esp. for the final chunk)
                    if ch == NCH - 1:
                        h0 = hsl.start
                        nc.scalar.activation(o[:, h0:h0 + 256], ps[:, 0:256],
                                             IDENT_FN, bias=bout_sb[ot])
                        nc.vector.tensor_scalar_add(
                            o[:, h0 + 256:h0 + 512], ps[:, 256:512],
                            bout_sb[ot])
                    elif ot % 2 == 1:
                        nc.scalar.activation(o[:, hsl], ps[:], IDENT_FN,
                                             bias=bout_sb[ot])
                    else:
                        nc.vector.tensor_scalar_add(o[:, hsl], ps[:],
                                                    bout_sb[ot])
                    if paired and ch % 2 == 1:
                        nc.sync.dma_start(
                            out_d[ot * P:(ot + 1) * P,
                                  (ch - 1) * 512:(ch + 1) * 512],
                            o[:],
                        )
                    elif not paired:
                        nc.sync.dma_start(
                            out_d[ot * P:(ot + 1) * P,
                                  ch * 512:(ch + 1) * 512],
                            o[:, 0:512],
                        )

    nc.compile()
    return nc


def _get_nc():
    global _CACHED_NC
    if _CACHED_NC is None:
        _CACHED_NC = _build()
    return _CACHED_NC


def _get_runner():
    """Memoized PJRT runner: jax.jit-compiled once, reused across kernel()
    calls (run_bass_kernel_spmd rebuilds the jit closure every call, which
    forces a ~minute-long recompile)."""
    global _CACHED_RUNNER
    if _CACHED_RUNNER is not None:
        return _CACHED_RUNNER

    import jax
    from jax.sharding import Mesh, PartitionSpec
    from jax.experimental.shard_map import shard_map
    import concourse.mybir as mybir_
    from concourse.bass2jax import (
        _bass_exec_p,
        install_neuronx_cc_hook,
        partition_id_tensor,
    )

    nc = _get_nc()
    install_neuronx_cc_hook()

    partition_name = (
        nc.partition_id_tensor.name if nc.partition_id_tensor else None
    )
    in_names = []
    out_names = []
    out_avals = []
    out_shapes = []
    for alloc in nc.m.functions[0].allocations:
        if not isinstance(alloc, mybir_.MemoryLocationSet):
            continue
        name = alloc.memorylocations[0].name
        if alloc.kind == "ExternalInput":
            if name != partition_name:
                in_names.append(name)
        elif alloc.kind == "ExternalOutput":
            shape = tuple(alloc.tensor_shape)
            dtype = mybir_.dt.np(alloc.dtype)
            out_names.append(name)
            out_avals.append(jax.core.ShapedArray(shape, dtype))
            out_shapes.append((shape, dtype))
    n_params = len(in_names)
    n_outs = len(out_names)
    all_names = tuple(in_names + out_names)
    if partition_name is not None:
        all_names = all_names + (partition_name,)
    donate = tuple(range(n_params, n_params + n_outs))

    def _body(*args):
        operands = list(args)
        if partition_name is not None:
            operands.append(partition_id_tensor())
        outs = _bass_exec_p.bind(
            *operands,
            out_avals=tuple(out_avals),
            in_names=all_names,
            out_names=tuple(out_names),
            lowering_input_output_aliases=(),
            sim_require_finite=True,
            sim_require_nnan=True,
            nc=nc,
        )
        return tuple(outs)

    devices = jax.devices()[:B]
    mesh = Mesh(np.asarray(devices), ("core",))
    sharded = jax.jit(
        shard_map(
            _body,
            mesh=mesh,
            in_specs=(PartitionSpec("core"),) * (n_params + n_outs),
            out_specs=(PartitionSpec("core"),) * n_outs,
            check_rep=False,
        ),
        donate_argnums=donate,
        keep_unused=True,
    )

    def run(in_maps):
        concat_in = [
            np.concatenate([np.asarray(m[k]) for m in in_maps], axis=0)
            for k in in_names
        ]
        concat_zeros = [
            np.zeros((B * s[0], *s[1:]), dt) for (s, dt) in out_shapes
        ]
        out_arrs = sharded(*concat_in, *concat_zeros)
        return [
            {
                k: np.asarray(out_arrs[i]).reshape(B, *out_shapes[i][0])[c]
                for i, k in enumerate(out_names)
            }
            for c in range(B)
        ]

    _CACHED_RUNNER = run
    return run


def kernel(f_m, f_n, Wq, Wkv, Wout, bout, trace=False):
    f_m = np.asarray(f_m, dtype=np.float32)
    f_n = np.asarray(f_n, dtype=np.float32)
    Wq = np.asarray(Wq, dtype=np.float32)
    Wkv = np.asarray(Wkv, dtype=np.float32)
    Wout = np.asarray(Wout, dtype=np.float32)
    bout = np.asarray(bout, dtype=np.float32)

    b, c, h, w = f_m.shape
    nc = _get_nc()
    bf = ml_dtypes.bfloat16
    fm_b = np.ascontiguousarray(f_m.reshape(b, C, NN).astype(bf))
    fn_b = np.ascontiguousarray(f_n.reshape(b, C, NN).astype(bf))
    wqt = np.ascontiguousarray(Wq.T.astype(bf))
    wkt = np.ascontiguousarray(Wkv[:C].T.astype(bf))
    wv = np.ascontiguousarray(Wkv[C:].astype(bf))
    woutt = np.ascontiguousarray(Wout.T.astype(bf))
    cst = np.ascontiguousarray(
        np.concatenate(
            [np.eye(P, dtype=np.float32), _DMASK, bout.reshape(CT, P).T],
            axis=1,
        ).astype(np.float32)
    )
    in_maps = [
        {
            "f_mb": fm_b[i],
            "f_nb": fn_b[i],
            "WqT": wqt,
            "WkT": wkt,
            "Wv": wv,
            "WoutT": woutt,
            "cst": cst,
        }
        for i in range(b)
    ]
    if trace:
        res = run_bass_kernel_spmd(
            nc, in_maps, core_ids=list(range(B)), trace=True
        )
        kernel.last_results = res
        results = res.results
    else:
        results = _get_runner()(in_maps)
    return np.stack(
        [r["out"].astype(np.float32).reshape(c, h, w) for r in results]
    )
